# revision 13
# baseline (speedup 1.0000x reference)
"""Trainium2 Bass kernel for CLIP attention pooling.

Reference computation (N=4096, D=1024, fp32):
    q = x @ Wq.T + bq
    k = x @ Wk.T + bk
    attn = softmax(q @ k.T, axis=-1)
    out = attn @ x

Math notes:
  * scores = q @ k.T = q @ Wk @ x.T + (q.bk) 1^T. The (q.bk) term is
    constant along the softmax axis, so bk never needs to be computed.
  * q @ Wk = x @ (Wq.T @ Wk) + bq @ Wk: both projections fold into one
    matrix M = Wq.T @ Wk and a row c = bq @ Wk (host-precomputed).
  * Per core (512 query rows):
        tT = M^T . xs^T + c          [D, 512]   (transposed layout)
        S  = t . x^T                 [512, 4096]
        P  = softmax(S)              (online, running-max)
        out = P @ x                  [512, 1024]
  * The whole score path runs in fp16 (10-bit mantissa ~ fp32r's 11);
    scores accumulate in fp32 PSUM. E is bf16, out is fp32.

Schedule:
  * phase A: M chunks stream on the sync HWDGE ring, xs chunks on the
    scalar ring (independent trigger FIFOs, half-chunk granularity so
    the e-loop chases arrivals); ~40 tiny identity matmuls warm the PE
    p-state while the first chunks land. e-outer over 8 PSUM banks; the
    bias c enters through the PSUM->SBUF copies (DVE tensor_scalar_add
    / ACT Identity-with-bias), not matmuls.
  * key-chunk rotation: each core processes key chunks in rotated order
    [c, c+1, ...]; its own query slice xTs IS rotated chunk 0 and is
    already in SBUF when phase B starts (zero DMA wait). xTb and xb are
    host-rotated to match; output rows are queries, so unaffected.
  * phase B: online softmax. Per (i, chunk): 8 accumulating matmuls
    into a PSUM bank, DVE keeps a running negated max straight out of
    PSUM, ACT applies exp(PSUM - runmax) into bf16 E with accum_out
    collecting per-chunk partial sums. No S buffer exists. Chunks 6-7
    run i0/i1-first so their finalize chain (c_k = exp(m_k - m_final),
    Z = sum z_k c_k, g_k = c_k/Z -> per-chunk diag(g) tiles) completes
    ~2 PE groups before B's last matmul; i3's final group sits on a
    dedicated PSUM bank (via pad tiles) so its exp can be deferred into
    phase C's queues without gating anything.
  * phase C pass 0 (i0/i1): "transposes" are plain matmuls over x
    resident in SBUF.
    E_tile @ diag(g) - softmax normalization and running-max correction
    ride the mandatory transpose for free. Two jt per fp32 pst bank,
    one PSUM->SBUF copy per pair (DVE/ACT alternating), LOOKP=2 pairs
    of lookahead. Deferred phase-B work and pass-0 drains are popped
    into the queues between pairs.
  * phase C pass 1 (i2/i3): same structure; the accumulator tiles are
    shared across passes and pass 0's drains are emitted interleaved
    into pass 1's first pairs, before the banks are reused.
  * x (bf16) is fully resident in SBUF: host-pretransposed to
    [128, 32, 1024] so two large contiguous DMAs load it behind the
    phase-B stream, aliasing the released weight pools.
  * outputs: each 128-row block is copied from two PSUM banks into one
    [128, 1024] tile (DVE + ACT in parallel) and leaves via a single
    full-width DMA, alternating sync/scalar rings; the last pass leaves
    only 1MB for the tail.
"""

import os
from contextlib import ExitStack

import numpy as np
import ml_dtypes

import concourse.bass as bass
import concourse.mybir as mybir
import concourse.tile as tile
from concourse import bacc
from concourse.bass_utils import run_bass_kernel_spmd
from concourse.masks import make_identity

N, D = 4096, 1024
NCORES = 8
R = N // NCORES  # 512 query rows per core
PT = 128  # partition tile
EC = D // PT  # 8 contraction chunks of the model dim
IT = R // PT  # 4 query tiles per core
JC = N // 512  # 8 key chunks of 512
JT = N // PT  # 32 key tiles of 128

F32 = mybir.dt.float32
F32R = mybir.dt.float32r
F16 = mybir.dt.float16
BF16 = mybir.dt.bfloat16
AX = mybir.AxisListType
AF = mybir.ActivationFunctionType
ALU = mybir.AluOpType

PASSES = ((0, 1), (2,), (3,))


def _emit(nc: bass.Bass, tc: tile.TileContext, aps: dict):
    xTb, xTs, mw, cw, xb, out = (
        aps["xTb"], aps["xTs"], aps["mw"], aps["cw"],
        aps["xb"], aps["out"],
    )

    with ExitStack() as big:
        persist = big.enter_context(tc.tile_pool(name="persist", bufs=1))

        # warmup source: a plain memset tile (~100ns on gpsimd) so the PE
        # p-state ramp can start without waiting for the iota-based identity.
        wsrc = persist.tile([PT, PT], BF16)
        nc.gpsimd.memset(wsrc, 0.0)
        ident = persist.tile([PT, PT], BF16)
        make_identity(nc, ident)
        c_sb = persist.tile([PT, EC], F32)
        fence_sb = persist.tile([PT, 8], F16)

        tT_sb = persist.tile([PT, EC, R], F16)
        E_bf = [persist.tile([PT, N], BF16, name=f"E{i}") for i in range(IT)]
        nmk = [persist.tile([PT, JC], F32, name=f"nmk{i}") for i in range(IT)]
        tmx = [persist.tile([PT, JC], F32, name=f"tmx{i}") for i in range(IT)]
        zpart = [persist.tile([PT, JC], F32, name=f"zp{i}") for i in range(IT)]
        ck = [persist.tile([PT, JC], F32, name=f"ck{i}") for i in range(IT)]
        gk = [persist.tile([PT, JC], F32, name=f"gk{i}") for i in range(IT)]
        zsum = [persist.tile([PT, 1], F32, name=f"z{i}") for i in range(IT)]
        rz = [persist.tile([PT, 1], F32, name=f"rz{i}") for i in range(IT)]
        diag = persist.tile([PT, IT, JC, PT], BF16)

        # opened before wpool so its addresses never overlap the weights;
        # the early stream triggers can then issue during phase A.
        xtpool = big.enter_context(tc.tile_pool(name="xtpool", bufs=4))
        xtjs = {}
        for j in range(1, JC):
            xtjs[j] = xtpool.tile([PT, EC, 512], F16, tag="xtj", name="xtj")

        # xs in its own pool (opened after xtpool so pools unwind LIFO):
        # stream position 0 of phase B reads it directly - each core's query
        # slice IS its own key chunk - so it is only released after that;
        # the xb buffer then aliases it + wpool.
        xspool_cm = tc.tile_pool(name="xspool", bufs=1)
        xspool = xspool_cm.__enter__()
        xts_sb = xspool.tile([PT, EC, R], F16)

        # ---- Phase A: tT = M^T.xs^T + c  (transposed layout)
        with ExitStack() as pha:
            wpool = pha.enter_context(tc.tile_pool(name="wpool", bufs=1))
            apsum = pha.enter_context(tc.tile_pool(name="apsum", bufs=1, space="PSUM"))

            m_sb = wpool.tile([PT, EC, D], F16)

            m_r = mw.rearrange("(t p) d -> p t d", p=PT)
            xTs_r = xTs.rearrange("(t p) i -> p t i", p=PT)
            # M rides the sync HWDGE ring, xs + bias the scalar ring: the
            # trigger FIFOs are independent and the SDMA engines round-robin
            # between them. Chunk 0 of the phase-B stream is slotted in
            # before the last two M chunks: phase A's compute tail covers it.
            nc.sync.dma_start(m_sb[:, 0, 0:PT], m_r[:, 0, 0:PT])
            nc.scalar.dma_start(xts_sb[:, 0, 0:256], xTs_r[:, 0, 0:256])
            nc.scalar.dma_start(xts_sb[:, 0, 256:512], xTs_r[:, 0, 256:512])
            nc.sync.dma_start(m_sb[:, 0, PT:D], m_r[:, 0, PT:D])
            nc.scalar.dma_start(xts_sb[:, 1, :], xTs_r[:, 1, :])
            nc.scalar.dma_start(c_sb, cw)
            for e in range(1, EC):
                nc.sync.dma_start(m_sb[:, e, 0:512], m_r[:, e, 0:512])
                nc.sync.dma_start(m_sb[:, e, 512:D], m_r[:, e, 512:D])
            for e in range(2, EC):
                nc.scalar.dma_start(xts_sb[:, e, :], xTs_r[:, e, :])

            tps = [
                apsum.tile([PT, R], F32, tag=f"tp{d}", name=f"tp{d}")
                for d in range(EC)
            ]
            # PE warm-up only while the first DMA chunks are in flight
            # (~1.3us): the real phase-A matmuls then continue the p-state
            # ramp doing useful work at the reduced clock (the e-loop chases
            # the M/xs streams and stays stall-free even at half clock).
            # Results are clobbered by the first start=True matmul.
            for _ in range(14):
                nc.tensor.matmul(
                    tps[0][:, 0:PT], wsrc, wsrc, start=True, stop=True
                )
            # e-outer for chunks 0-4 (chasing the M/xs streams), then each
            # bank finishes its last 3 chunks d-by-d so the 8 stops stagger
            # ~0.65us apart instead of bunching in A's last round. Each
            # bank's PSUM->SBUF copy (bias c folded in) is issued right at
            # its stop, split into DVE+ACT halves running in parallel: the
            # copies fully chase the stops and B's first group (which
            # inherits ALL copies' deps via group-level merging) starts
            # ~0.4us after A's last matmul instead of ~1.3us.
            ESPLIT = 5
            for e in range(ESPLIT):
                for d in range(EC):
                    nc.tensor.matmul(
                        tps[d],
                        m_sb[:, e, d * PT : (d + 1) * PT],
                        xts_sb[:, e, :],
                        start=(e == 0),
                        stop=False,
                    )
            for d in range(EC):
                for e in range(ESPLIT, EC):
                    nc.tensor.matmul(
                        tps[d],
                        m_sb[:, e, d * PT : (d + 1) * PT],
                        xts_sb[:, e, :],
                        start=False,
                        stop=(e == EC - 1),
                    )
                # full-width copy, engines alternating by d: half-splits of
                # one d-slice get serialized DVE->ACT by the dep tracker.
                if d % 2 == 0:
                    nc.vector.tensor_scalar_add(
                        tT_sb[:, d, :], tps[d], c_sb[:, d : d + 1]
                    )
                else:
                    nc.scalar.activation(
                        tT_sb[:, d, :], tps[d], func=AF.Identity,
                        bias=c_sb[:, d : d + 1],
                    )
            # gpsimd fence: a dummy read of the last M chunk holds the xb
            # stream triggers (gpsimd ring) until the M stream has fully
            # landed, so xb's 8MB can't starve phase A's weight stream.
            nc.gpsimd.tensor_copy(fence_sb, m_sb[:, EC - 1, 0:8])

        # ---- Phase B: S chunks in PSUM + online softmax straight to E.
        def softmax_step(ps, i, j):
            if j == 0:
                nc.vector.reduce_max(
                    out=nmk[i][:, 0:1], in_=ps, axis=AX.X, negate=True
                )
            else:
                nc.vector.reduce_max(
                    out=tmx[i][:, j : j + 1], in_=ps, axis=AX.X, negate=True
                )
                nc.vector.tensor_tensor(
                    out=nmk[i][:, j : j + 1],
                    in0=nmk[i][:, j - 1 : j],
                    in1=tmx[i][:, j : j + 1],
                    op=ALU.min,
                )
            nc.scalar.activation(
                out=E_bf[i][:, j * 512 : (j + 1) * 512],
                in_=ps,
                func=AF.Exp,
                bias=nmk[i][:, j : j + 1],
                scale=1.0,
                accum_out=zpart[i][:, j : j + 1],
            )

        def finalize_pair(ia, ib):
            # c_k = exp(m_k - m_last), Z = sum z_k c_k, g = c_k/Z; then the
            # per-chunk diag(g) tiles, k-ordered round-robin across DVE/ACT
            # so both i-tiles' early-k diags finish first, in parallel.
            for i in (ia, ib):
                nc.scalar.activation(
                    out=ck[i],
                    in_=nmk[i],
                    func=AF.Exp,
                    bias=nmk[i][:, JC - 1 : JC],
                    scale=-1.0,
                )
            for i in (ia, ib):
                nc.vector.tensor_tensor(
                    out=gk[i], in0=zpart[i], in1=ck[i], op=ALU.mult
                )
            for i in (ia, ib):
                nc.vector.reduce_sum(out=zsum[i], in_=gk[i], axis=AX.X)
            for i in (ia, ib):
                nc.vector.reciprocal(rz[i], zsum[i])
            for i in (ia, ib):
                nc.vector.tensor_scalar_mul(gk[i], ck[i], rz[i])
            for k in range(JC):
                dve_i = ia if k % 2 == 0 else ib
                act_i = ib if k % 2 == 0 else ia
                nc.vector.tensor_scalar_mul(
                    diag[:, dve_i, k, :], ident, gk[dve_i][:, k : k + 1]
                )
                nc.scalar.activation(
                    diag[:, act_i, k, :],
                    ident,
                    func=AF.Copy,
                    scale=gk[act_i][:, k : k + 1],
                )

        bpend = []
        with ExitStack() as phb:
            spsum = phb.enter_context(tc.tile_pool(name="spsum", bufs=4, space="PSUM"))
            padpool = phb.enter_context(
                tc.tile_pool(name="padpool", bufs=1, space="PSUM")
            )
            def mm_group(ps, i, xtj):
                for d in range(EC):
                    nc.tensor.matmul(
                        ps,
                        tT_sb[:, d, i * PT : (i + 1) * PT],
                        xtj[:, d, :],
                        start=(d == 0),
                        stop=(d == EC - 1),
                    )

            for j in range(JC - 2):
                xtj = xts_sb if j == 0 else xtjs[j]
                if j > 0:
                    nc.sync.dma_start(xtj, xTb[j])
                for i in range(IT):
                    ps = spsum.tile([PT, 512], F32, tag="Sp", name="Sp")
                    mm_group(ps, i, xtj)
                    softmax_step(ps, i, j)
                if j == 0:
                    xspool_cm.__exit__(None, None, None)

            # Chunks 6-7 are processed i0/i1-first so their finalize chain
            # (which gates phase C's first transposes) completes ~2 PE groups
            # before B's compute ends. i3's last group goes to a dedicated
            # PSUM bank so its deferred exp gates nothing in phase C.
            nc.sync.dma_start(xtjs[JC - 2], xTb[JC - 2])
            nc.sync.dma_start(xtjs[JC - 1], xTb[JC - 1])
            xt6, xt7 = xtjs[JC - 2], xtjs[JC - 1]

            def subchain(i):
                # gk = ck * (1/Z); only the k0 diag is built inline (k1+ ride
                # the deferred stream, needed two jt-pairs later). i1's build
                # goes to ACT so it runs parallel to this DVE chain.
                nc.vector.tensor_tensor(
                    out=gk[i], in0=zpart[i], in1=ck[i], op=ALU.mult
                )
                nc.vector.reduce_sum(out=zsum[i], in_=gk[i], axis=AX.X)
                nc.vector.reciprocal(rz[i], zsum[i])
                nc.vector.tensor_scalar_mul(gk[i], ck[i], rz[i])
                for k in (0, 1):
                    if i == 0:
                        nc.vector.tensor_scalar_mul(
                            diag[:, 0, k, :], ident, gk[0][:, k : k + 1]
                        )
                    else:
                        nc.scalar.activation(
                            diag[:, 1, k, :], ident, func=AF.Copy,
                            scale=gk[1][:, k : k + 1],
                        )

            for i in (0, 1):
                ps = spsum.tile([PT, 512], F32, tag="Sp", name="Sp")
                mm_group(ps, i, xt6)
                softmax_step(ps, i, JC - 2)
            ps70 = spsum.tile([PT, 512], F32, tag="Sp", name="Sp")
            mm_group(ps70, 0, xt7)
            ps71 = spsum.tile([PT, 512], F32, tag="Sp", name="Sp")
            mm_group(ps71, 1, xt7)
            softmax_step(ps70, 0, JC - 1)
            nc.scalar.activation(
                out=ck[0], in_=nmk[0], func=AF.Exp,
                bias=nmk[0][:, JC - 1 : JC], scale=-1.0,
            )
            softmax_step(ps71, 1, JC - 1)
            nc.scalar.activation(
                out=ck[1], in_=nmk[1], func=AF.Exp,
                bias=nmk[1][:, JC - 1 : JC], scale=-1.0,
            )
            subchain(0)
            subchain(1)
            for i in (2, 3):
                ps = spsum.tile([PT, 512], F32, tag="Sp", name="Sp")
                mm_group(ps, i, xt6)
                softmax_step(ps, i, JC - 2)
            ps72 = spsum.tile([PT, 512], F32, tag="Sp", name="Sp")
            mm_group(ps72, 2, xt7)
            softmax_step(ps72, 2, JC - 1)
            # pad tiles reserve banks 4-6 (never written) so the final i3
            # group lands on bank 7, which phase C never reallocates; its
            # exp can then be deferred into phase C's queues safely.
            for pb in range(3):
                padpool.tile([PT, 512], F32, tag=f"pad{pb}", name=f"pad{pb}")
            ps73 = padpool.tile([PT, 512], F32, tag="Spz", name="Spz")
            mm_group(ps73, 3, xt7)

            def red3min3():
                nc.vector.reduce_max(
                    out=tmx[3][:, JC - 1 : JC], in_=ps73, axis=AX.X, negate=True
                )
                nc.vector.tensor_tensor(
                    out=nmk[3][:, JC - 1 : JC], in0=nmk[3][:, JC - 2 : JC - 1],
                    in1=tmx[3][:, JC - 1 : JC], op=ALU.min,
                )

            def exp3():
                nc.scalar.activation(
                    out=E_bf[3][:, (JC - 1) * 512 : JC * 512],
                    in_=ps73, func=AF.Exp,
                    bias=nmk[3][:, JC - 1 : JC], scale=1.0,
                    accum_out=zpart[3][:, JC - 1 : JC],
                )

            def dgk01(k):
                def emit():
                    nc.vector.tensor_scalar_mul(
                        diag[:, 0, k, :], ident, gk[0][:, k : k + 1]
                    )
                    nc.scalar.activation(
                        diag[:, 1, k, :], ident, func=AF.Copy,
                        scale=gk[1][:, k : k + 1],
                    )
                return emit

            def fin23a():
                for i in (2, 3):
                    nc.scalar.activation(
                        out=ck[i], in_=nmk[i], func=AF.Exp,
                        bias=nmk[i][:, JC - 1 : JC], scale=-1.0,
                    )
                for i in (2, 3):
                    nc.vector.tensor_tensor(
                        out=gk[i], in0=zpart[i], in1=ck[i], op=ALU.mult
                    )
                for i in (2, 3):
                    nc.vector.reduce_sum(out=zsum[i], in_=gk[i], axis=AX.X)
                for i in (2, 3):
                    nc.vector.reciprocal(rz[i], zsum[i])
                for i in (2, 3):
                    nc.vector.tensor_scalar_mul(gk[i], ck[i], rz[i])

            def dg23(k0, k1):
                def emit():
                    for k in range(k0, k1):
                        nc.vector.tensor_scalar_mul(
                            diag[:, 2, k, :], ident, gk[2][:, k : k + 1]
                        )
                        nc.scalar.activation(
                            diag[:, 3, k, :], ident, func=AF.Copy,
                            scale=gk[3][:, k : k + 1],
                        )
                return emit

            bpend.append(red3min3)
            bpend.append(exp3)
            for k in range(2, JC):
                bpend.append(dgk01(k))
            bpend.append(fin23a)
            bpend.append(dg23(0, 4))
            bpend.append(dg23(4, JC))

        # ---- Phase C: out = P @ x with x fully resident in SBUF.
        # xb reuses the phase-A weight pool's address range. Its triggers sit
        # on the GPSIMD ring: the gpsimd engine queue is empty after phase A,
        # so the transfers start at ~A-end on a third ring, leaving the sync
        # ring entirely to the phase-B xtj stream (no queue contention).
        xbpool = big.enter_context(tc.tile_pool(name="xbpool", bufs=1))
        xb_sb = xbpool.tile([PT, JT, D], BF16)
        for qh in range(4):
            nc.gpsimd.dma_start(
                xb_sb[:, qh * 8 : (qh + 1) * 8, :], xb[:, qh * 8 : (qh + 1) * 8, :]
            )
        etpool = big.enter_context(tc.tile_pool(name="etpool", bufs=4))
        ocopy = big.enter_context(tc.tile_pool(name="ocopy", bufs=2))
        # opsum allocated before tpsum: oacc lands on banks 0-3, whose last
        # phase-B exps clear 1.5-5us before B's end; tpsum gets the pad banks
        # (never written) plus ps73's bank (freed by the qv1-popped exp3).
        opsum = big.enter_context(tc.tile_pool(name="opsum", bufs=1, space="PSUM"))
        tpsum = big.enter_context(tc.tile_pool(name="tpsum", bufs=3, space="PSUM"))
        # One pass per i-tile so each pass's output drain (copy + 512KB DMA)
        # overlaps the next pass's ~15.6us of compute; only i3's drain is
        # exposed at the very end, split across both rings. Accumulator banks
        # alternate by pass parity so pass p's start=True matmuls never wait
        # on pass p-1's drain (only on p-2's, ~15us stale).
        oacc = {
            (par, dn): opsum.tile(
                [PT, 512], F32, tag=f"o{par}_{dn}", name=f"o{par}_{dn}"
            )
            for par in range(2)
            for dn in range(2)
        }

        def drain_item(i, par):
            # both dn halves into one [PT, D] tile (DVE + ACT in parallel),
            # then a single full-width DMA with 4KB rows, rings alternating.
            def emit():
                ot = ocopy.tile([PT, D], F32, tag="ot", name="ot")
                nc.vector.tensor_copy(ot[:, 0:512], oacc[(par, 0)])
                nc.scalar.activation(ot[:, 512:D], oacc[(par, 1)], func=AF.Copy)
                eng = nc.sync if i % 2 == 0 else nc.scalar
                eng.dma_start(out[i * PT : (i + 1) * PT, :], ot)

            return emit

        QV = JT // 4  # 8 groups of 4 jt per pass
        LOOKP = 3
        pending = bpend
        for i in range(IT):
            par = i % 2
            ets = {}
            for qv in range(QV + LOOKP):
                if qv < QV:
                    # "transpose" = E_tile.T @ diag(g): per-row softmax scale
                    # applied for free by the mandatory transpose. Four jt per
                    # pst bank -> one PSUM->SBUF copy per group (DVE/ACT
                    # alternating), LOOKP=2 groups of lookahead.
                    pst = tpsum.tile([PT, 512], F32, tag="tp", name="pst")
                    for s in range(4):
                        jt = 4 * qv + s
                        nc.tensor.matmul(
                            pst[:, s * PT : (s + 1) * PT],
                            E_bf[i][:, jt * PT : (jt + 1) * PT],
                            diag[:, i, jt // 4, :],
                            start=True,
                            stop=True,
                            skip_group_check=True,
                        )
                    et = etpool.tile([PT, 512], BF16, tag="et", name="et")
                    if qv % 2 == 0:
                        nc.vector.tensor_copy(et, pst)
                    else:
                        nc.scalar.activation(et, pst, func=AF.Copy)
                    ets[qv % 4] = et
                    # pops start at qv 1 so group 0's et copy isn't queued
                    # behind the deferred DVE work; three per group so the
                    # diag(g) tiles for chunk k are always built before the
                    # qv=k transposes that read them.
                    if qv >= 1:
                        for _ in range(3):
                            if pending:
                                pending.pop(0)()
                if qv >= LOOKP:
                    q = qv - LOOKP
                    for s in range(4):
                        jt = 4 * q + s
                        for dn in range(2):
                            nc.tensor.matmul(
                                oacc[(par, dn)],
                                ets[q % 4][:, s * PT : (s + 1) * PT],
                                xb_sb[:, jt, dn * 512 : (dn + 1) * 512],
                                start=(jt == 0),
                                stop=(jt == JT - 1),
                            )
            if i < IT - 1:
                pending = pending + [drain_item(i, par)]
            else:
                # final drain: dn halves leave on BOTH rings as soon as each
                # accumulator bank stops, halving the exposed tail DMA.
                ot = ocopy.tile([PT, D], F32, tag="ot", name="ot")
                nc.vector.tensor_copy(ot[:, 0:512], oacc[(par, 0)])
                nc.sync.dma_start(out[i * PT : (i + 1) * PT, 0:512], ot[:, 0:512])
                nc.scalar.activation(ot[:, 512:D], oacc[(par, 1)], func=AF.Copy)
                nc.scalar.dma_start(
                    out[i * PT : (i + 1) * PT, 512:D], ot[:, 512:D]
                )


def build():
    nc = bacc.Bacc(
        "TRN2",
        target_bir_lowering=False,
        debug=False,
        enable_asserts=False,
        num_devices=NCORES,
    )
    aps = {
        "xTb": nc.dram_tensor("xTb", [JC, PT, EC, 512], F16, kind="ExternalInput").ap(),
        "xTs": nc.dram_tensor("xTs", [D, R], F16, kind="ExternalInput").ap(),
        "mw": nc.dram_tensor("mw", [D, D], F16, kind="ExternalInput").ap(),
        "cw": nc.dram_tensor("cw", [PT, EC], F32, kind="ExternalInput").ap(),
        "xb": nc.dram_tensor("xb", [PT, JT, D], BF16, kind="ExternalInput").ap(),
        "out": nc.dram_tensor("out", [R, D], F32, kind="ExternalOutput").ap(),
    }
    with tile.TileContext(nc) as tc:
        _emit(nc, tc, aps)
    nc.compile()
    return nc


_NC_CACHE = None
LAST_RESULTS = None


def _get_nc():
    global _NC_CACHE
    if _NC_CACHE is None:
        _NC_CACHE = build()
    return _NC_CACHE


def make_in_maps(x, Wq, bq, Wk):
    x = np.ascontiguousarray(np.asarray(x, dtype=np.float32))
    xT = np.ascontiguousarray(x.T).astype(np.float16)
    # xTb[j, p, e, n] = xT[e*128 + p, j*512 + n]: per-(j,p) contiguous 16KB
    # blocks so the phase-B stream DMAs at full descriptor size.
    xTb = np.ascontiguousarray(
        xT.reshape(EC, PT, JC, 512).transpose(2, 1, 0, 3)
    )
    wk64 = np.asarray(Wk, dtype=np.float64)
    mw = np.ascontiguousarray(
        (np.asarray(Wq, dtype=np.float64).T @ wk64).astype(np.float16)
    )
    # cw[p, e] = c[e*128 + p]: per-partition bias column for the tT copies.
    cw = np.ascontiguousarray(
        (np.asarray(bq, dtype=np.float64) @ wk64)
        .astype(np.float32)
        .reshape(EC, PT)
        .T
    )
    xb = x.astype(ml_dtypes.bfloat16)
    in_maps = []
    for c in range(NCORES):
        # Each core processes key chunks in rotated order [c, c+1, ..]: its
        # own query slice xTs doubles as stream position 0 (already in SBUF
        # when phase B starts), so xTb and xb are rotated to match. The
        # rotation permutes softmax terms and P@x rows consistently; the
        # output rows (queries) are unaffected.
        in_maps.append(
            {
                "xTb": np.ascontiguousarray(
                    np.concatenate([xTb[c:], xTb[:c]], axis=0)
                ),
                "xTs": np.ascontiguousarray(xT[:, c * R : (c + 1) * R]),
                "mw": mw,
                "cw": cw,
                "xb": np.ascontiguousarray(
                    np.roll(xb, -512 * c, axis=0)
                    .reshape(JT, PT, D)
                    .transpose(1, 0, 2)
                ),
            }
        )
    return in_maps


def kernel(x, Wq, bq, Wk, bk):
    # bk only shifts each score row by a constant, which softmax cancels.
    del bk
    in_maps = make_in_maps(x, Wq, bq, Wk)
    nc = _get_nc()
    kwargs = {}
    if os.environ.get("K_TRACE_DIR"):
        import tempfile

        kwargs["tmpdir"] = tempfile.mkdtemp(dir=os.environ["K_TRACE_DIR"])
    res = run_bass_kernel_spmd(nc, in_maps, core_ids=list(range(NCORES)), **kwargs)
    global LAST_RESULTS
    LAST_RESULTS = res
    return np.concatenate(
        [np.asarray(res.results[c]["out"], dtype=np.float32) for c in range(NCORES)],
        axis=0,
    )

